# revision 15
# baseline (speedup 1.0000x reference)
"""Trainium2 Bass kernel for the MessagePassingLayer problem.

Reference computation (per particle row n, K=32 neighbors, F=4, W=256):
    f      = silu(differences @ W_f1 + b_f1)        # [N,K,W]
    filter = f @ W_f2 + b_f2                        # [N,K,W]
    h_nb   = h_neighbors @ W_nb + b_nb              # [N,K,W]
    msg    = sum_k(filter * h_nb)                   # [N,W]
    out    = silu(h_center @ W_c + b_c + msg)       # [N,W]

Strategy: data-parallel over the leading N axis across 8 cores (no
cross-device communication).  On each core everything is computed in a
"feature-major" layout (features on SBUF partitions, edges along the free
dimension) so that:
  - weight matrices are the stationary matmul operands,
  - all four biases ride for free on ops we need anyway (ACT bias slots
    and the scalar_tensor_tensor fused scalar),
  - the k-reduction is a grouped free-axis tensor_reduce.
Matmuls run as float32r (full-rate PE pass over fp32 bytes).
"""

import sys
from contextlib import ExitStack

sys.path.insert(0, "/opt/trn_rl_repo")

import numpy as np

import concourse.bass as bass
import concourse.tile as tile
from concourse import bacc, dve_ops, mybir
from concourse.bass_utils import run_bass_kernel_spmd
from concourse.dve_spec import AluOp as _AluOp
from concourse.dve_spec import C0, Spec, Src0, Src1
from concourse.dve_spec import _has_src1
from concourse.dve_spec import lower as _dve_lower
from concourse.dve_spec import scan as _dve_scan
from concourse.dve_uop import DveOpSpec as _DveOpSpec

# Problem shape (hardcoded per the task contract).
N, K, F, W = 32768, 32, 4, 256
NCORES = 8
NC_ROWS = N // NCORES          # 4096 particle rows per core
E = NC_ROWS * K                # 131072 edges per core
TE = 512                       # edge-tile size (free dim)
NT = E // TE                   # 256 edge tiles per core
NB = 512                       # center-block rows
NBLK = NC_ROWS // NB           # 8 center blocks per core
TPB = NT // NBLK               # 32 edge tiles per center block
GRP = TE // K                  # 16 particle rows per edge tile
XSUP = 8                       # edge tiles per x-DMA (1 MiB transfers)

F32 = mybir.dt.float32
F32R = mybir.dt.float32r
F16 = mybir.dt.float16
DT_NP = np.float32

AX = mybir.AxisListType
OP = mybir.AluOpType
AF = mybir.ActivationFunctionType


def _r(ap):
    """View an fp32 AP as float32r for full-rate PE matmuls."""
    return ap.bitcast(F32R)


# The walrus LDW-dedup pass removes the implicit LDWEIGHTS of a matmul whose
# stationary operand is identical to the previous one in the PE stream; the
# kernel pairs same-weight matmuls specifically to expose this.
ENABLE_LDW_OPT = False


def _patch_ldw_opt():
    from concourse import bass_utils as _bu
    if getattr(_bu, "_ldwopt_patched", False):
        return
    orig = _bu.run_command

    def run_command_ldwopt(cmd, **kw):
        if isinstance(cmd, list):
            cmd = ["--enable-ldw-opt=true" if c == "--enable-ldw-opt=false"
                   else c for c in cmd]
        return orig(cmd, **kw)

    _bu.run_command = run_command_ldwopt
    _bu._ldwopt_patched = True


if ENABLE_LDW_OPT:
    _patch_ldw_opt()


def _mul_cumsum_op():
    """Custom DVE op: out[p, :] = cumsum((in0[p, :] + s0[p]) * in1[p, :]).

    Fuses the b_f2 bias, the gating multiply, and the running k-sum into a
    single 1x DVE pass; per-row message sums are recovered afterwards from
    strided differences of the cumulative sums.
    """
    name = "MUL_CUMSUM_ANT"
    for o in dve_ops.OPS:
        if o.name == name:
            return o
    body = _dve_scan(_AluOp.ADD, (Src0 + C0) * Src1)

    def _ref(in0, in1, c0, c1, c2):
        t = (in0.astype(np.float32) + c0) * in1.astype(np.float32)
        return np.cumsum(t, axis=-1, dtype=np.float32)

    spec = Spec(body=body, reference=_ref)
    row = dve_ops._CUSTOM_DVE_ROW_BASE + len(dve_ops.OPS)
    assert row < 0x20, "custom-DVE row table full"
    dve_ops._SUB_OPCODE_FOR_NAME[name] = row
    shas = {}
    for ver in ("v3", "v4"):
        try:
            s = _DveOpSpec(name=name, opcode=row,
                           uops=_dve_lower(spec, ver=ver),
                           rd1_en=_has_src1(spec))
            shas[ver] = s.sha(ver)
        except Exception:
            pass
    op = dve_ops.DveOp(name, spec, subdim=False, uops_sha=shas)
    dve_ops.OPS.append(op)
    dve_ops.CUSTOM_DVE_SPECS[name] = spec
    return op


_MUL_CUMSUM = _mul_cumsum_op()


def _build():
    nc = bacc.Bacc("TRN2")

    xt = nc.declare_dram_parameter("xt", [2, 128, E], F16, isOutput=False)
    dt_ = nc.declare_dram_parameter("dt", [F, E], F16, isOutput=False)
    ct = nc.declare_dram_parameter("ct", [2, 128, NC_ROWS], F16, isOutput=False)
    wf1 = nc.declare_dram_parameter("wf1", [F, W], F16, isOutput=False)
    wf2 = nc.declare_dram_parameter("wf2", [2, 128, W], F16, isOutput=False)
    wnb = nc.declare_dram_parameter("wnb", [2, 128, W], F16, isOutput=False)
    wc = nc.declare_dram_parameter("wc", [2, 128, W], F16, isOutput=False)
    bf1 = nc.declare_dram_parameter("bf1", [2, 128, 1], F32, isOutput=False)
    bf2 = nc.declare_dram_parameter("bf2", [2, 128, 1], F32, isOutput=False)
    bnb = nc.declare_dram_parameter("bnb", [2, 128, 1], F32, isOutput=False)
    bc = nc.declare_dram_parameter("bc", [2, 128, 1], F32, isOutput=False)
    out = nc.declare_dram_parameter("out_t", [2, 128, NC_ROWS], F32, isOutput=True)

    with tile.TileContext(nc) as tc, ExitStack() as ctx:
        # PE warm-up: dependency-free matmuls on uninitialized scratch keep
        # the PE-HAM activity window busy while the first real loads land,
        # so real matmuls start at the full 2.4 GHz clock.
        with tc.tile_pool(name="warmp", bufs=1, space="PSUM") as wp, \
                tc.tile_pool(name="warms", bufs=1) as ws:
            junk = ws.tile([128, TE], F16, tag="junk")
            jw = ws.tile([128, 128], F16, tag="jw")
            pj = wp.tile([128, TE], F32, tag="pj")
            nc.vector.memset(junk[:], 0.0)
            nc.vector.memset(jw[:], 0.0)
            for _ in range(72):
                nc.tensor.matmul(pj[:], jw[:], junk[:], start=True, stop=True)

        const = ctx.enter_context(tc.tile_pool(name="const", bufs=1))

        wf1_t = const.tile([F, W], F16, tag="wf1")
        nc.sync.dma_start(wf1_t[:], wf1[:])
        wf2_t = []
        wnb_t = []
        wc_t = []
        for v in range(2):
            w2 = const.tile([128, W], F16, tag=f"wf2{v}", name=f"wf2_{v}")
            nc.sync.dma_start(w2[:], wf2[v])
            wf2_t.append(w2)
            wn = const.tile([128, W], F16, tag=f"wnb{v}", name=f"wnb_{v}")
            nc.sync.dma_start(wn[:], wnb[v])
            wnb_t.append(wn)
            wcv = const.tile([128, W], F16, tag=f"wc{v}", name=f"wc_{v}")
            nc.sync.dma_start(wcv[:], wc[v])
            wc_t.append(wcv)
        bias_t = {}
        for nm, src in (("bf1", bf1), ("bf2", bf2), ("bnb", bnb), ("bc", bc)):
            for m in range(2):
                b = const.tile([128, 1], F32, tag=f"{nm}{m}", name=f"{nm}_{m}")
                nc.sync.dma_start(b[:], src[m])
                bias_t[(nm, m)] = b

        xp = ctx.enter_context(tc.tile_pool(name="xp", bufs=3))
        dp = ctx.enter_context(tc.tile_pool(name="dp", bufs=2))
        fp = ctx.enter_context(tc.tile_pool(name="fp", bufs=4))
        bp = ctx.enter_context(tc.tile_pool(name="bp", bufs=4))
        gp = ctx.enter_context(tc.tile_pool(name="gp", bufs=4))
        mp = ctx.enter_context(tc.tile_pool(name="mp", bufs=2))
        cp = ctx.enter_context(tc.tile_pool(name="cp", bufs=2))
        op_ = ctx.enter_context(tc.tile_pool(name="op", bufs=2))
        # PSUM: pf covers the f1 and f2 matmul outputs (shared rotation),
        # pn covers the neighbor and center projections.  2 tags x 2 bufs
        # per pool = exactly the 8 banks.
        pf = ctx.enter_context(tc.tile_pool(name="pf", bufs=2, space="PSUM"))
        pn = ctx.enter_context(tc.tile_pool(name="pn", bufs=2, space="PSUM"))

        for j in range(NBLK):
            msg = [mp.tile([128, NB], F32, tag=f"msg{m}", name=f"msg{m}_{j}")
                   for m in range(2)]
            # one 64 KiB diff load per half-block (16 tiles)
            d_sb = dp.tile([F, TPB * TE // 2], F16, tag="d", name=f"d_{j}a")
            nc.sync.dma_start(
                d_sb[:], dt_[:, j * TPB * TE:(j * TPB + TPB // 2) * TE])
            for sup in range(TPB // XSUP):
                if sup == TPB // XSUP // 2:
                    d_sb = dp.tile([F, TPB * TE // 2], F16, tag="d",
                                   name=f"d_{j}b")
                    nc.sync.dma_start(
                        d_sb[:],
                        dt_[:, (j * TPB + TPB // 2) * TE:(j + 1) * TPB * TE])
                # 1 MiB x loads covering XSUP edge tiles per chunk
                xs = []
                t0 = j * TPB + sup * XSUP
                esup = slice(t0 * TE, (t0 + XSUP) * TE)
                for m in range(2):
                    x_ = xp.tile([128, XSUP * TE], F16, tag=f"x{m}",
                                 name=f"x{m}_{t0}")
                    if j == 0 and sup == 0:
                        # split the very first load so the first tile pair's
                        # data arrives without waiting for the full 1 MiB
                        q = XSUP * TE // 4
                        for h in range(4):
                            nc.sync.dma_start(
                                x_[:, h * q:(h + 1) * q],
                                xt[m, :, t0 * TE + h * q:t0 * TE + (h + 1) * q])
                    else:
                        nc.sync.dma_start(x_[:], xt[m, :, esup])
                    xs.append(x_)
                for tp in range(0, XSUP, 2):
                    # tiles processed in pairs so matmuls sharing a stationary
                    # operand sit adjacent in the PE stream (LDW dedup)
                    tms = [sup * XSUP + tp, sup * XSUP + tp + 1]
                    ts_ = [j * TPB + tm for tm in tms]
                    els = [slice((tp + i) * TE, (tp + i + 1) * TE)
                           for i in range(2)]
                    dls = [slice((tm % (TPB // 2)) * TE,
                                 (tm % (TPB // 2) + 1) * TE) for tm in tms]

                    p1s = {}
                    for m in range(2):
                        ms = slice(m * 128, (m + 1) * 128)
                        for i in range(2):
                            p1 = pf.tile([128, TE], F32, tag=f"pf{m}",
                                         name=f"p1_{ts_[i]}_{m}")
                            nc.tensor.matmul(p1[:], wf1_t[:, ms],
                                             d_sb[:, dls[i]],
                                             start=True, stop=True)
                            p1s[(i, m)] = p1
                    fts = {}
                    for i in range(2):
                        for m in range(2):
                            f_ = fp.tile([128, TE], F16, tag=f"f{m}",
                                         name=f"f_{ts_[i]}_{m}")
                            nc.scalar.activation(f_[:], p1s[(i, m)][:],
                                                 AF.Silu,
                                                 bias=bias_t[("bf1", m)][:])
                            fts[(i, m)] = f_

                    p2s = {}
                    pns = {}
                    for m in range(2):
                        for i in range(2):
                            p2s[(i, m)] = pf.tile(
                                [128, TE], F32, tag=f"pf{m}",
                                name=f"p2_{ts_[i]}_{m}")
                            pns[(i, m)] = pn.tile(
                                [128, TE], F32, tag=f"pn{m}",
                                name=f"pn_{ts_[i]}_{m}")
                    for m in range(2):
                        ms = slice(m * 128, (m + 1) * 128)
                        for v in range(2):
                            for i in range(2):
                                nc.tensor.matmul(p2s[(i, m)][:],
                                                 wf2_t[v][:, ms],
                                                 fts[(i, v)][:],
                                                 start=(v == 0),
                                                 stop=(v == 1))
                        for v in range(2):
                            for i in range(2):
                                nc.tensor.matmul(pns[(i, m)][:],
                                                 wnb_t[v][:, ms],
                                                 xs[v][:, els[i]],
                                                 start=(v == 0),
                                                 stop=(v == 1))

                    for i in range(2):
                        tm = tms[i]
                        t = ts_[i]
                        for m in range(2):
                            p2 = p2s[(i, m)]
                            pnb = pns[(i, m)]
                            # B = h_nb + b_nb (PSUM -> SBUF move with fused
                            # bias); a quarter of the moves go to DVE to
                            # balance ACT vs DVE load.
                            b_ = bp.tile([128, TE], F32, tag=f"b{m}",
                                         name=f"b_{t}_{m}")
                            if m == 1 and t % 2 == 1:
                                nc.vector.tensor_scalar_add(
                                    b_[:], pnb[:], bias_t[("bnb", m)][:])
                            else:
                                nc.scalar.activation(
                                    b_[:], pnb[:], AF.Identity,
                                    bias=bias_t[("bnb", m)][:])
                            # g = cumsum_k((filter+b_f2)*B) in one DVE op
                            g = gp.tile([128, TE], F32, tag=f"g{m}",
                                        name=f"g_{t}_{m}")
                            nc.vector._custom_dve(
                                _MUL_CUMSUM, out=g[:], in0=p2[:], in1=b_[:],
                                s0=bias_t[("bf2", m)][:])
                            # per-row sums via strided cumsum differences
                            nc.vector.tensor_copy(
                                msg[m][:, tm * GRP:tm * GRP + 1],
                                g[:, K - 1:K])
                            nc.vector.tensor_sub(
                                msg[m][:, tm * GRP + 1:(tm + 1) * GRP],
                                g[:, 2 * K - 1::K], g[:, K - 1:TE - K:K])

            ns = slice(j * NB, (j + 1) * NB)
            c0 = cp.tile([128, NB], F16, tag="c0", name=f"c0_{j}")
            nc.sync.dma_start(c0[:], ct[0, :, ns])
            c1 = cp.tile([128, NB], F16, tag="c1", name=f"c1_{j}")
            nc.sync.dma_start(c1[:], ct[1, :, ns])
            for m in range(2):
                ms = slice(m * 128, (m + 1) * 128)
                pc = pn.tile([128, NB], F32, tag=f"pn{m}", name=f"pc_{j}_{m}")
                nc.tensor.matmul(pc[:], wc_t[0][:, ms], c0[:],
                                 start=True, stop=False)
                nc.tensor.matmul(pc[:], wc_t[1][:, ms], c1[:],
                                 start=False, stop=True)
                s = bp.tile([128, NB], F32, tag=f"b{m}", name=f"s_{j}_{m}")
                nc.vector.tensor_add(s[:], pc[:], msg[m][:])
                o = op_.tile([128, NB], F32, tag=f"o{m}", name=f"o_{j}_{m}")
                nc.scalar.activation(o[:], s[:], AF.Silu, bias=bias_t[("bc", m)][:])
                nc.sync.dma_start(out[m, :, ns], o[:])

    nc.compile()
    return nc


_NC_CACHE = None
_last_in_maps = None


def _get_nc():
    global _NC_CACHE
    if _NC_CACHE is None:
        _NC_CACHE = _build()
    return _NC_CACHE


def kernel(h_center, h_neighbors, differences, W_f1, b_f1, W_f2, b_f2,
           W_nb, b_nb, W_c, b_c):
    h_center = np.asarray(h_center, dtype=np.float32)
    h_neighbors = np.asarray(h_neighbors, dtype=np.float32)
    differences = np.asarray(differences, dtype=np.float32)

    wf1 = np.ascontiguousarray(np.asarray(W_f1, np.float16))    # [4, W]
    wf2 = np.ascontiguousarray(np.asarray(W_f2, np.float16)).reshape(2, 128, W)
    wnb = np.ascontiguousarray(np.asarray(W_nb, np.float16)).reshape(2, 128, W)
    wc = np.ascontiguousarray(np.asarray(W_c, np.float16)).reshape(2, 128, W)
    bf1 = np.asarray(b_f1, np.float32).reshape(2, 128, 1)
    bf2 = np.asarray(b_f2, np.float32).reshape(2, 128, 1)
    bnb = np.asarray(b_nb, np.float32).reshape(2, 128, 1)
    bc = np.asarray(b_c, np.float32).reshape(2, 128, 1)

    in_maps = []
    for c in range(NCORES):
        rs = slice(c * NC_ROWS, (c + 1) * NC_ROWS)
        xt = np.ascontiguousarray(
            h_neighbors[rs].reshape(E, W).T.astype(np.float16)).reshape(2, 128, E)
        dt_ = np.ascontiguousarray(differences[rs].reshape(E, F).T.astype(np.float16))
        ct = np.ascontiguousarray(h_center[rs].T.astype(np.float16)).reshape(2, 128, NC_ROWS)
        in_maps.append(dict(xt=xt, dt=dt_, ct=ct, wf1=wf1, wf2=wf2, wnb=wnb,
                            wc=wc, bf1=bf1, bf2=bf2, bnb=bnb, bc=bc))

    global _last_in_maps
    _last_in_maps = in_maps
    nc = _get_nc()
    res = run_bass_kernel_spmd(nc, in_maps, list(range(NCORES)))

    out = np.empty((N, W), np.float32)
    for c in range(NCORES):
        rs = slice(c * NC_ROWS, (c + 1) * NC_ROWS)
        out[rs] = res.results[c]["out_t"].reshape(W, NC_ROWS).T
    return out


# revision 16
# speedup vs baseline: 1.0129x; 1.0129x over previous
"""Trainium2 Bass kernel for the MessagePassingLayer problem.

Reference computation (per particle row n, K=32 neighbors, F=4, W=256):
    f      = silu(differences @ W_f1 + b_f1)        # [N,K,W]
    filter = f @ W_f2 + b_f2                        # [N,K,W]
    h_nb   = h_neighbors @ W_nb + b_nb              # [N,K,W]
    msg    = sum_k(filter * h_nb)                   # [N,W]
    out    = silu(h_center @ W_c + b_c + msg)       # [N,W]

Strategy: data-parallel over the leading N axis across 8 cores (no
cross-device communication).  On each core everything is computed in a
"feature-major" layout (features on SBUF partitions, edges along the free
dimension) so that:
  - weight matrices are the stationary matmul operands,
  - all four biases ride for free on ops we need anyway (ACT bias slots
    and the scalar_tensor_tensor fused scalar),
  - the k-reduction is a grouped free-axis tensor_reduce.
Matmuls run as float32r (full-rate PE pass over fp32 bytes).
"""

import sys
from contextlib import ExitStack

sys.path.insert(0, "/opt/trn_rl_repo")

import numpy as np

import concourse.bass as bass
import concourse.tile as tile
from concourse import bacc, dve_ops, mybir
from concourse.bass_utils import run_bass_kernel_spmd
from concourse.dve_spec import AluOp as _AluOp
from concourse.dve_spec import C0, Spec, Src0, Src1
from concourse.dve_spec import _has_src1
from concourse.dve_spec import lower as _dve_lower
from concourse.dve_spec import scan as _dve_scan
from concourse.dve_uop import DveOpSpec as _DveOpSpec

# Problem shape (hardcoded per the task contract).
N, K, F, W = 32768, 32, 4, 256
NCORES = 8
NC_ROWS = N // NCORES          # 4096 particle rows per core
E = NC_ROWS * K                # 131072 edges per core
TE = 512                       # edge-tile size (free dim)
NT = E // TE                   # 256 edge tiles per core
NB = 512                       # center-block rows
NBLK = NC_ROWS // NB           # 8 center blocks per core
TPB = NT // NBLK               # 32 edge tiles per center block
GRP = TE // K                  # 16 particle rows per edge tile
XSUP = 8                       # edge tiles per x-DMA (1 MiB transfers)

F32 = mybir.dt.float32
F32R = mybir.dt.float32r
F16 = mybir.dt.float16
DT_NP = np.float32

AX = mybir.AxisListType
OP = mybir.AluOpType
AF = mybir.ActivationFunctionType


def _r(ap):
    """View an fp32 AP as float32r for full-rate PE matmuls."""
    return ap.bitcast(F32R)


# The walrus LDW-dedup pass removes the implicit LDWEIGHTS of a matmul whose
# stationary operand is identical to the previous one in the PE stream; the
# kernel pairs same-weight matmuls specifically to expose this.
ENABLE_LDW_OPT = False


def _patch_ldw_opt():
    from concourse import bass_utils as _bu
    if getattr(_bu, "_ldwopt_patched", False):
        return
    orig = _bu.run_command

    def run_command_ldwopt(cmd, **kw):
        if isinstance(cmd, list):
            cmd = ["--enable-ldw-opt=true" if c == "--enable-ldw-opt=false"
                   else c for c in cmd]
        return orig(cmd, **kw)

    _bu.run_command = run_command_ldwopt
    _bu._ldwopt_patched = True


if ENABLE_LDW_OPT:
    _patch_ldw_opt()


def _mul_cumsum_op():
    """Custom DVE op: out[p, :] = cumsum((in0[p, :] + s0[p]) * in1[p, :]).

    Fuses the b_f2 bias, the gating multiply, and the running k-sum into a
    single 1x DVE pass; per-row message sums are recovered afterwards from
    strided differences of the cumulative sums.
    """
    name = "MUL_CUMSUM_ANT"
    for o in dve_ops.OPS:
        if o.name == name:
            return o
    body = _dve_scan(_AluOp.ADD, (Src0 + C0) * Src1)

    def _ref(in0, in1, c0, c1, c2):
        t = (in0.astype(np.float32) + c0) * in1.astype(np.float32)
        return np.cumsum(t, axis=-1, dtype=np.float32)

    spec = Spec(body=body, reference=_ref)
    row = dve_ops._CUSTOM_DVE_ROW_BASE + len(dve_ops.OPS)
    assert row < 0x20, "custom-DVE row table full"
    dve_ops._SUB_OPCODE_FOR_NAME[name] = row
    shas = {}
    for ver in ("v3", "v4"):
        try:
            s = _DveOpSpec(name=name, opcode=row,
                           uops=_dve_lower(spec, ver=ver),
                           rd1_en=_has_src1(spec))
            shas[ver] = s.sha(ver)
        except Exception:
            pass
    op = dve_ops.DveOp(name, spec, subdim=False, uops_sha=shas)
    dve_ops.OPS.append(op)
    dve_ops.CUSTOM_DVE_SPECS[name] = spec
    return op


_MUL_CUMSUM = _mul_cumsum_op()


def _build():
    nc = bacc.Bacc("TRN2")

    xt = nc.declare_dram_parameter("xt", [2, 128, E], F16, isOutput=False)
    dt_ = nc.declare_dram_parameter("dt", [F, E], F16, isOutput=False)
    ct = nc.declare_dram_parameter("ct", [2, 128, NC_ROWS], F16, isOutput=False)
    wf1 = nc.declare_dram_parameter("wf1", [F, W], F16, isOutput=False)
    wf2 = nc.declare_dram_parameter("wf2", [2, 128, W], F16, isOutput=False)
    wnb = nc.declare_dram_parameter("wnb", [2, 128, W], F16, isOutput=False)
    wc = nc.declare_dram_parameter("wc", [2, 128, W], F16, isOutput=False)
    bf1 = nc.declare_dram_parameter("bf1", [2, 128, 1], F32, isOutput=False)
    bf2 = nc.declare_dram_parameter("bf2", [2, 128, 1], F32, isOutput=False)
    bnb = nc.declare_dram_parameter("bnb", [2, 128, 1], F32, isOutput=False)
    bc = nc.declare_dram_parameter("bc", [2, 128, 1], F32, isOutput=False)
    out = nc.declare_dram_parameter("out_t", [2, 128, NC_ROWS], F32, isOutput=True)

    with tile.TileContext(nc) as tc, ExitStack() as ctx:
        const = ctx.enter_context(tc.tile_pool(name="const", bufs=1))

        wf1_t = const.tile([F, W], F16, tag="wf1")
        nc.sync.dma_start(wf1_t[:], wf1[:])
        wf2_t = []
        wnb_t = []
        wc_t = []
        for v in range(2):
            w2 = const.tile([128, W], F16, tag=f"wf2{v}", name=f"wf2_{v}")
            nc.sync.dma_start(w2[:], wf2[v])
            wf2_t.append(w2)
            wn = const.tile([128, W], F16, tag=f"wnb{v}", name=f"wnb_{v}")
            nc.sync.dma_start(wn[:], wnb[v])
            wnb_t.append(wn)
            wcv = const.tile([128, W], F16, tag=f"wc{v}", name=f"wc_{v}")
            nc.sync.dma_start(wcv[:], wc[v])
            wc_t.append(wcv)
        bias_t = {}
        for nm, src in (("bf1", bf1), ("bf2", bf2), ("bnb", bnb), ("bc", bc)):
            for m in range(2):
                b = const.tile([128, 1], F32, tag=f"{nm}{m}", name=f"{nm}_{m}")
                nc.sync.dma_start(b[:], src[m])
                bias_t[(nm, m)] = b

        xp = ctx.enter_context(tc.tile_pool(name="xp", bufs=3))
        dp = ctx.enter_context(tc.tile_pool(name="dp", bufs=2))
        fp = ctx.enter_context(tc.tile_pool(name="fp", bufs=4))
        bp = ctx.enter_context(tc.tile_pool(name="bp", bufs=4))
        gp = ctx.enter_context(tc.tile_pool(name="gp", bufs=4))
        mp = ctx.enter_context(tc.tile_pool(name="mp", bufs=2))
        cp = ctx.enter_context(tc.tile_pool(name="cp", bufs=2))
        op_ = ctx.enter_context(tc.tile_pool(name="op", bufs=2))
        # PSUM: pf covers the f1 and f2 matmul outputs (shared rotation),
        # pn covers the neighbor and center projections.  2 tags x 2 bufs
        # per pool = exactly the 8 banks.
        pf = ctx.enter_context(tc.tile_pool(name="pf", bufs=2, space="PSUM"))
        pn = ctx.enter_context(tc.tile_pool(name="pn", bufs=2, space="PSUM"))

        for j in range(NBLK):
            msg = [mp.tile([128, NB], F32, tag=f"msg{m}", name=f"msg{m}_{j}")
                   for m in range(2)]
            # one 64 KiB diff load per half-block (16 tiles)
            d_sb = dp.tile([F, TPB * TE // 2], F16, tag="d", name=f"d_{j}a")
            nc.sync.dma_start(
                d_sb[:], dt_[:, j * TPB * TE:(j * TPB + TPB // 2) * TE])
            for sup in range(TPB // XSUP):
                if sup == TPB // XSUP // 2:
                    d_sb = dp.tile([F, TPB * TE // 2], F16, tag="d",
                                   name=f"d_{j}b")
                    nc.sync.dma_start(
                        d_sb[:],
                        dt_[:, (j * TPB + TPB // 2) * TE:(j + 1) * TPB * TE])
                # 1 MiB x loads covering XSUP edge tiles per chunk
                xs = []
                t0 = j * TPB + sup * XSUP
                esup = slice(t0 * TE, (t0 + XSUP) * TE)
                for m in range(2):
                    x_ = xp.tile([128, XSUP * TE], F16, tag=f"x{m}",
                                 name=f"x{m}_{t0}")
                    if j == 0 and sup == 0:
                        # split the very first load so the first tile pair's
                        # data arrives without waiting for the full 1 MiB
                        q = XSUP * TE // 4
                        for h in range(4):
                            nc.sync.dma_start(
                                x_[:, h * q:(h + 1) * q],
                                xt[m, :, t0 * TE + h * q:t0 * TE + (h + 1) * q])
                    else:
                        nc.sync.dma_start(x_[:], xt[m, :, esup])
                    xs.append(x_)
                for tp in range(0, XSUP, 2):
                    # tiles processed in pairs so matmuls sharing a stationary
                    # operand sit adjacent in the PE stream (LDW dedup)
                    tms = [sup * XSUP + tp, sup * XSUP + tp + 1]
                    ts_ = [j * TPB + tm for tm in tms]
                    els = [slice((tp + i) * TE, (tp + i + 1) * TE)
                           for i in range(2)]
                    dls = [slice((tm % (TPB // 2)) * TE,
                                 (tm % (TPB // 2) + 1) * TE) for tm in tms]

                    p1s = {}
                    for m in range(2):
                        ms = slice(m * 128, (m + 1) * 128)
                        for i in range(2):
                            p1 = pf.tile([128, TE], F32, tag=f"pf{m}",
                                         name=f"p1_{ts_[i]}_{m}")
                            nc.tensor.matmul(p1[:], wf1_t[:, ms],
                                             d_sb[:, dls[i]],
                                             start=True, stop=True)
                            p1s[(i, m)] = p1
                    fts = {}
                    for i in range(2):
                        for m in range(2):
                            f_ = fp.tile([128, TE], F16, tag=f"f{m}",
                                         name=f"f_{ts_[i]}_{m}")
                            nc.scalar.activation(f_[:], p1s[(i, m)][:],
                                                 AF.Silu,
                                                 bias=bias_t[("bf1", m)][:])
                            fts[(i, m)] = f_

                    p2s = {}
                    pns = {}
                    for m in range(2):
                        for i in range(2):
                            p2s[(i, m)] = pf.tile(
                                [128, TE], F32, tag=f"pf{m}",
                                name=f"p2_{ts_[i]}_{m}")
                            pns[(i, m)] = pn.tile(
                                [128, TE], F32, tag=f"pn{m}",
                                name=f"pn_{ts_[i]}_{m}")
                    for m in range(2):
                        ms = slice(m * 128, (m + 1) * 128)
                        for v in range(2):
                            for i in range(2):
                                nc.tensor.matmul(p2s[(i, m)][:],
                                                 wf2_t[v][:, ms],
                                                 fts[(i, v)][:],
                                                 start=(v == 0),
                                                 stop=(v == 1))
                        for v in range(2):
                            for i in range(2):
                                nc.tensor.matmul(pns[(i, m)][:],
                                                 wnb_t[v][:, ms],
                                                 xs[v][:, els[i]],
                                                 start=(v == 0),
                                                 stop=(v == 1))

                    for i in range(2):
                        tm = tms[i]
                        t = ts_[i]
                        for m in range(2):
                            p2 = p2s[(i, m)]
                            pnb = pns[(i, m)]
                            # B = h_nb + b_nb (PSUM -> SBUF move with fused
                            # bias); a quarter of the moves go to DVE to
                            # balance ACT vs DVE load.
                            b_ = bp.tile([128, TE], F32, tag=f"b{m}",
                                         name=f"b_{t}_{m}")
                            if m == 1 and t % 2 == 1:
                                nc.vector.tensor_scalar_add(
                                    b_[:], pnb[:], bias_t[("bnb", m)][:])
                            else:
                                nc.scalar.activation(
                                    b_[:], pnb[:], AF.Identity,
                                    bias=bias_t[("bnb", m)][:])
                            # g = cumsum_k((filter+b_f2)*B) in one DVE op
                            g = gp.tile([128, TE], F32, tag=f"g{m}",
                                        name=f"g_{t}_{m}")
                            nc.vector._custom_dve(
                                _MUL_CUMSUM, out=g[:], in0=p2[:], in1=b_[:],
                                s0=bias_t[("bf2", m)][:])
                            # per-row sums via strided cumsum differences
                            nc.vector.tensor_copy(
                                msg[m][:, tm * GRP:tm * GRP + 1],
                                g[:, K - 1:K])
                            nc.vector.tensor_sub(
                                msg[m][:, tm * GRP + 1:(tm + 1) * GRP],
                                g[:, 2 * K - 1::K], g[:, K - 1:TE - K:K])

            ns = slice(j * NB, (j + 1) * NB)
            c0 = cp.tile([128, NB], F16, tag="c0", name=f"c0_{j}")
            nc.sync.dma_start(c0[:], ct[0, :, ns])
            c1 = cp.tile([128, NB], F16, tag="c1", name=f"c1_{j}")
            nc.sync.dma_start(c1[:], ct[1, :, ns])
            for m in range(2):
                ms = slice(m * 128, (m + 1) * 128)
                pc = pn.tile([128, NB], F32, tag=f"pn{m}", name=f"pc_{j}_{m}")
                nc.tensor.matmul(pc[:], wc_t[0][:, ms], c0[:],
                                 start=True, stop=False)
                nc.tensor.matmul(pc[:], wc_t[1][:, ms], c1[:],
                                 start=False, stop=True)
                s = bp.tile([128, NB], F32, tag=f"b{m}", name=f"s_{j}_{m}")
                nc.vector.tensor_add(s[:], pc[:], msg[m][:])
                o = op_.tile([128, NB], F32, tag=f"o{m}", name=f"o_{j}_{m}")
                nc.scalar.activation(o[:], s[:], AF.Silu, bias=bias_t[("bc", m)][:])
                nc.sync.dma_start(out[m, :, ns], o[:])

    nc.compile()
    return nc


_NC_CACHE = None
_last_in_maps = None


def _get_nc():
    global _NC_CACHE
    if _NC_CACHE is None:
        _NC_CACHE = _build()
    return _NC_CACHE


def kernel(h_center, h_neighbors, differences, W_f1, b_f1, W_f2, b_f2,
           W_nb, b_nb, W_c, b_c):
    h_center = np.asarray(h_center, dtype=np.float32)
    h_neighbors = np.asarray(h_neighbors, dtype=np.float32)
    differences = np.asarray(differences, dtype=np.float32)

    wf1 = np.ascontiguousarray(np.asarray(W_f1, np.float16))    # [4, W]
    wf2 = np.ascontiguousarray(np.asarray(W_f2, np.float16)).reshape(2, 128, W)
    wnb = np.ascontiguousarray(np.asarray(W_nb, np.float16)).reshape(2, 128, W)
    wc = np.ascontiguousarray(np.asarray(W_c, np.float16)).reshape(2, 128, W)
    bf1 = np.asarray(b_f1, np.float32).reshape(2, 128, 1)
    bf2 = np.asarray(b_f2, np.float32).reshape(2, 128, 1)
    bnb = np.asarray(b_nb, np.float32).reshape(2, 128, 1)
    bc = np.asarray(b_c, np.float32).reshape(2, 128, 1)

    in_maps = []
    for c in range(NCORES):
        rs = slice(c * NC_ROWS, (c + 1) * NC_ROWS)
        xt = np.ascontiguousarray(
            h_neighbors[rs].reshape(E, W).T.astype(np.float16)).reshape(2, 128, E)
        dt_ = np.ascontiguousarray(differences[rs].reshape(E, F).T.astype(np.float16))
        ct = np.ascontiguousarray(h_center[rs].T.astype(np.float16)).reshape(2, 128, NC_ROWS)
        in_maps.append(dict(xt=xt, dt=dt_, ct=ct, wf1=wf1, wf2=wf2, wnb=wnb,
                            wc=wc, bf1=bf1, bf2=bf2, bnb=bnb, bc=bc))

    global _last_in_maps
    _last_in_maps = in_maps
    nc = _get_nc()
    res = run_bass_kernel_spmd(nc, in_maps, list(range(NCORES)))

    out = np.empty((N, W), np.float32)
    for c in range(NCORES):
        rs = slice(c * NC_ROWS, (c + 1) * NC_ROWS)
        out[rs] = res.results[c]["out_t"].reshape(W, NC_ROWS).T
    return out


# revision 18
# speedup vs baseline: 1.0133x; 1.0004x over previous
"""Trainium2 Bass kernel for the MessagePassingLayer problem.

Reference computation (per particle row n, K=32 neighbors, F=4, W=256):
    f      = silu(differences @ W_f1 + b_f1)        # [N,K,W]
    filter = f @ W_f2 + b_f2                        # [N,K,W]
    h_nb   = h_neighbors @ W_nb + b_nb              # [N,K,W]
    msg    = sum_k(filter * h_nb)                   # [N,W]
    out    = silu(h_center @ W_c + b_c + msg)       # [N,W]

Strategy: data-parallel over the leading N axis across 8 cores (no
cross-device communication).  On each core everything is computed in a
"feature-major" layout (features on SBUF partitions, edges along the free
dimension) so that:
  - weight matrices are the stationary matmul operands,
  - all four biases ride for free on ops we need anyway (ACT bias slots
    and the scalar_tensor_tensor fused scalar),
  - the k-reduction is a grouped free-axis tensor_reduce.
Matmuls run as float32r (full-rate PE pass over fp32 bytes).
"""

import sys
from contextlib import ExitStack

sys.path.insert(0, "/opt/trn_rl_repo")

import numpy as np

import concourse.bass as bass
import concourse.tile as tile
from concourse import bacc, dve_ops, mybir
from concourse.bass_utils import run_bass_kernel_spmd
from concourse.dve_spec import AluOp as _AluOp
from concourse.dve_spec import C0, Spec, Src0, Src1
from concourse.dve_spec import _has_src1
from concourse.dve_spec import lower as _dve_lower
from concourse.dve_spec import scan as _dve_scan
from concourse.dve_uop import DveOpSpec as _DveOpSpec

# Problem shape (hardcoded per the task contract).
N, K, F, W = 32768, 32, 4, 256
NCORES = 8
NC_ROWS = N // NCORES          # 4096 particle rows per core
E = NC_ROWS * K                # 131072 edges per core
TE = 512                       # edge-tile size (free dim)
NT = E // TE                   # 256 edge tiles per core
NB = 512                       # center-block rows
NBLK = NC_ROWS // NB           # 8 center blocks per core
TPB = NT // NBLK               # 32 edge tiles per center block
GRP = TE // K                  # 16 particle rows per edge tile
XSUP = 8                       # edge tiles per x-DMA (1 MiB transfers)

F32 = mybir.dt.float32
F32R = mybir.dt.float32r
F16 = mybir.dt.float16
DT_NP = np.float32

AX = mybir.AxisListType
OP = mybir.AluOpType
AF = mybir.ActivationFunctionType


def _r(ap):
    """View an fp32 AP as float32r for full-rate PE matmuls."""
    return ap.bitcast(F32R)


# The walrus LDW-dedup pass removes the implicit LDWEIGHTS of a matmul whose
# stationary operand is identical to the previous one in the PE stream; the
# kernel pairs same-weight matmuls specifically to expose this.
ENABLE_LDW_OPT = False


def _patch_ldw_opt():
    from concourse import bass_utils as _bu
    if getattr(_bu, "_ldwopt_patched", False):
        return
    orig = _bu.run_command

    def run_command_ldwopt(cmd, **kw):
        if isinstance(cmd, list):
            cmd = ["--enable-ldw-opt=true" if c == "--enable-ldw-opt=false"
                   else c for c in cmd]
        return orig(cmd, **kw)

    _bu.run_command = run_command_ldwopt
    _bu._ldwopt_patched = True


if ENABLE_LDW_OPT:
    _patch_ldw_opt()


def _mul_cumsum_op():
    """Custom DVE op: out[p, :] = cumsum((in0[p, :] + s0[p]) * in1[p, :]).

    Fuses the b_f2 bias, the gating multiply, and the running k-sum into a
    single 1x DVE pass; per-row message sums are recovered afterwards from
    strided differences of the cumulative sums.
    """
    name = "MUL_CUMSUM_ANT"
    for o in dve_ops.OPS:
        if o.name == name:
            return o
    body = _dve_scan(_AluOp.ADD, (Src0 + C0) * Src1)

    def _ref(in0, in1, c0, c1, c2):
        t = (in0.astype(np.float32) + c0) * in1.astype(np.float32)
        return np.cumsum(t, axis=-1, dtype=np.float32)

    spec = Spec(body=body, reference=_ref)
    row = dve_ops._CUSTOM_DVE_ROW_BASE + len(dve_ops.OPS)
    assert row < 0x20, "custom-DVE row table full"
    dve_ops._SUB_OPCODE_FOR_NAME[name] = row
    shas = {}
    for ver in ("v3", "v4"):
        try:
            s = _DveOpSpec(name=name, opcode=row,
                           uops=_dve_lower(spec, ver=ver),
                           rd1_en=_has_src1(spec))
            shas[ver] = s.sha(ver)
        except Exception:
            pass
    op = dve_ops.DveOp(name, spec, subdim=False, uops_sha=shas)
    dve_ops.OPS.append(op)
    dve_ops.CUSTOM_DVE_SPECS[name] = spec
    return op


_MUL_CUMSUM = _mul_cumsum_op()


def _build():
    nc = bacc.Bacc("TRN2")

    xt = nc.declare_dram_parameter("xt", [2, 128, E], F16, isOutput=False)
    dt_ = nc.declare_dram_parameter("dt", [F, E], F16, isOutput=False)
    ct = nc.declare_dram_parameter("ct", [2, 128, NC_ROWS], F16, isOutput=False)
    wf1 = nc.declare_dram_parameter("wf1", [F, W], F16, isOutput=False)
    wf2 = nc.declare_dram_parameter("wf2", [2, 128, W], F16, isOutput=False)
    wnb = nc.declare_dram_parameter("wnb", [2, 128, W], F16, isOutput=False)
    wc = nc.declare_dram_parameter("wc", [2, 128, W], F16, isOutput=False)
    bf1 = nc.declare_dram_parameter("bf1", [2, 128, 1], F32, isOutput=False)
    bf2 = nc.declare_dram_parameter("bf2", [2, 128, 1], F32, isOutput=False)
    bnb = nc.declare_dram_parameter("bnb", [2, 128, 1], F32, isOutput=False)
    bc = nc.declare_dram_parameter("bc", [2, 128, 1], F32, isOutput=False)
    out = nc.declare_dram_parameter("out_t", [2, 128, NC_ROWS], F32, isOutput=True)

    with tile.TileContext(nc) as tc, ExitStack() as ctx:
        const = ctx.enter_context(tc.tile_pool(name="const", bufs=1))

        wf1_t = const.tile([F, W], F16, tag="wf1")
        nc.sync.dma_start(wf1_t[:], wf1[:])
        wf2_t = []
        wnb_t = []
        wc_t = []
        for v in range(2):
            w2 = const.tile([128, W], F16, tag=f"wf2{v}", name=f"wf2_{v}")
            nc.sync.dma_start(w2[:], wf2[v])
            wf2_t.append(w2)
            wn = const.tile([128, W], F16, tag=f"wnb{v}", name=f"wnb_{v}")
            nc.sync.dma_start(wn[:], wnb[v])
            wnb_t.append(wn)
            wcv = const.tile([128, W], F16, tag=f"wc{v}", name=f"wc_{v}")
            nc.sync.dma_start(wcv[:], wc[v])
            wc_t.append(wcv)
        bias_t = {}
        for nm, src in (("bf1", bf1), ("bf2", bf2), ("bnb", bnb), ("bc", bc)):
            for m in range(2):
                b = const.tile([128, 1], F32, tag=f"{nm}{m}", name=f"{nm}_{m}")
                nc.sync.dma_start(b[:], src[m])
                bias_t[(nm, m)] = b

        xp = ctx.enter_context(tc.tile_pool(name="xp", bufs=3))
        dp = ctx.enter_context(tc.tile_pool(name="dp", bufs=2))
        fp = ctx.enter_context(tc.tile_pool(name="fp", bufs=4))
        bp = ctx.enter_context(tc.tile_pool(name="bp", bufs=4))
        gp = ctx.enter_context(tc.tile_pool(name="gp", bufs=4))
        mp = ctx.enter_context(tc.tile_pool(name="mp", bufs=2))
        cp = ctx.enter_context(tc.tile_pool(name="cp", bufs=2))
        op_ = ctx.enter_context(tc.tile_pool(name="op", bufs=2))
        # PSUM: pf covers the f1 and f2 matmul outputs (shared rotation),
        # pn covers the neighbor and center projections.  2 tags x 2 bufs
        # per pool = exactly the 8 banks.
        pf = ctx.enter_context(tc.tile_pool(name="pf", bufs=2, space="PSUM"))
        pn = ctx.enter_context(tc.tile_pool(name="pn", bufs=2, space="PSUM"))

        for j in range(NBLK):
            msg = [mp.tile([128, NB], F32, tag=f"msg{m}", name=f"msg{m}_{j}")
                   for m in range(2)]
            # one 64 KiB diff load per half-block (16 tiles)
            d_sb = dp.tile([F, TPB * TE // 2], F16, tag="d", name=f"d_{j}a")
            nc.sync.dma_start(
                d_sb[:], dt_[:, j * TPB * TE:(j * TPB + TPB // 2) * TE])
            for sup in range(TPB // XSUP):
                if sup == TPB // XSUP // 2:
                    d_sb = dp.tile([F, TPB * TE // 2], F16, tag="d",
                                   name=f"d_{j}b")
                    nc.sync.dma_start(
                        d_sb[:],
                        dt_[:, (j * TPB + TPB // 2) * TE:(j + 1) * TPB * TE])
                # 1 MiB x loads covering XSUP edge tiles per chunk
                xs = []
                t0 = j * TPB + sup * XSUP
                esup = slice(t0 * TE, (t0 + XSUP) * TE)
                for m in range(2):
                    x_ = xp.tile([128, XSUP * TE], F16, tag=f"x{m}",
                                 name=f"x{m}_{t0}")
                    if j == 0 and sup == 0:
                        # split the very first load so the first tile pair's
                        # data arrives without waiting for the full 1 MiB
                        q = XSUP * TE // 4
                        for h in range(4):
                            nc.sync.dma_start(
                                x_[:, h * q:(h + 1) * q],
                                xt[m, :, t0 * TE + h * q:t0 * TE + (h + 1) * q])
                    else:
                        nc.sync.dma_start(x_[:], xt[m, :, esup])
                    xs.append(x_)
                for tp in range(0, XSUP, 2):
                    # tiles processed in pairs so matmuls sharing a stationary
                    # operand sit adjacent in the PE stream (LDW dedup)
                    tms = [sup * XSUP + tp, sup * XSUP + tp + 1]
                    ts_ = [j * TPB + tm for tm in tms]
                    els = [slice((tp + i) * TE, (tp + i + 1) * TE)
                           for i in range(2)]
                    dls = [slice((tm % (TPB // 2)) * TE,
                                 (tm % (TPB // 2) + 1) * TE) for tm in tms]

                    p1s = {}
                    for m in range(2):
                        ms = slice(m * 128, (m + 1) * 128)
                        for i in range(2):
                            p1 = pf.tile([128, TE], F32, tag=f"pf{m}",
                                         name=f"p1_{ts_[i]}_{m}")
                            nc.tensor.matmul(p1[:], wf1_t[:, ms],
                                             d_sb[:, dls[i]],
                                             start=True, stop=True)
                            p1s[(i, m)] = p1
                    fts = {}
                    for i in range(2):
                        for m in range(2):
                            f_ = fp.tile([128, TE], F16, tag=f"f{m}",
                                         name=f"f_{ts_[i]}_{m}")
                            nc.scalar.activation(f_[:], p1s[(i, m)][:],
                                                 AF.Silu,
                                                 bias=bias_t[("bf1", m)][:])
                            fts[(i, m)] = f_

                    p2s = {}
                    pns = {}
                    for m in range(2):
                        for i in range(2):
                            p2s[(i, m)] = pf.tile(
                                [128, TE], F32, tag=f"pf{m}",
                                name=f"p2_{ts_[i]}_{m}")
                            pns[(i, m)] = pn.tile(
                                [128, TE], F32, tag=f"pn{m}",
                                name=f"pn_{ts_[i]}_{m}")
                    for m in range(2):
                        ms = slice(m * 128, (m + 1) * 128)
                        for v in range(2):
                            for i in range(2):
                                nc.tensor.matmul(pns[(i, m)][:],
                                                 wnb_t[v][:, ms],
                                                 xs[v][:, els[i]],
                                                 start=(v == 0),
                                                 stop=(v == 1))
                        for v in range(2):
                            for i in range(2):
                                nc.tensor.matmul(p2s[(i, m)][:],
                                                 wf2_t[v][:, ms],
                                                 fts[(i, v)][:],
                                                 start=(v == 0),
                                                 stop=(v == 1))

                    for i in range(2):
                        tm = tms[i]
                        t = ts_[i]
                        for m in range(2):
                            p2 = p2s[(i, m)]
                            pnb = pns[(i, m)]
                            # B = h_nb + b_nb (PSUM -> SBUF move with fused
                            # bias); one in four goes to DVE for balance.
                            b_ = bp.tile([128, TE], F32, tag=f"b{m}",
                                         name=f"b_{t}_{m}")
                            if m == 1 and t % 2 == 1:
                                nc.vector.tensor_scalar_add(
                                    b_[:], pnb[:], bias_t[("bnb", m)][:])
                            else:
                                nc.scalar.activation(
                                    b_[:], pnb[:], AF.Identity,
                                    bias=bias_t[("bnb", m)][:])
                            # g = cumsum_k((filter+b_f2)*B) in one DVE op
                            g = gp.tile([128, TE], F32, tag=f"g{m}",
                                        name=f"g_{t}_{m}")
                            nc.vector._custom_dve(
                                _MUL_CUMSUM, out=g[:], in0=p2[:], in1=b_[:],
                                s0=bias_t[("bf2", m)][:])
                            # per-row sums via strided cumsum differences
                            nc.vector.tensor_copy(
                                msg[m][:, tm * GRP:tm * GRP + 1],
                                g[:, K - 1:K])
                            nc.vector.tensor_sub(
                                msg[m][:, tm * GRP + 1:(tm + 1) * GRP],
                                g[:, 2 * K - 1::K], g[:, K - 1:TE - K:K])

            ns = slice(j * NB, (j + 1) * NB)
            c0 = cp.tile([128, NB], F16, tag="c0", name=f"c0_{j}")
            nc.sync.dma_start(c0[:], ct[0, :, ns])
            c1 = cp.tile([128, NB], F16, tag="c1", name=f"c1_{j}")
            nc.sync.dma_start(c1[:], ct[1, :, ns])
            for m in range(2):
                ms = slice(m * 128, (m + 1) * 128)
                pc = pn.tile([128, NB], F32, tag=f"pn{m}", name=f"pc_{j}_{m}")
                nc.tensor.matmul(pc[:], wc_t[0][:, ms], c0[:],
                                 start=True, stop=False)
                nc.tensor.matmul(pc[:], wc_t[1][:, ms], c1[:],
                                 start=False, stop=True)
                s = bp.tile([128, NB], F32, tag=f"b{m}", name=f"s_{j}_{m}")
                nc.vector.tensor_add(s[:], pc[:], msg[m][:])
                o = op_.tile([128, NB], F32, tag=f"o{m}", name=f"o_{j}_{m}")
                nc.scalar.activation(o[:], s[:], AF.Silu, bias=bias_t[("bc", m)][:])
                nc.sync.dma_start(out[m, :, ns], o[:])

    nc.compile()
    return nc


_NC_CACHE = None
_last_in_maps = None


def _get_nc():
    global _NC_CACHE
    if _NC_CACHE is None:
        _NC_CACHE = _build()
    return _NC_CACHE


def kernel(h_center, h_neighbors, differences, W_f1, b_f1, W_f2, b_f2,
           W_nb, b_nb, W_c, b_c):
    h_center = np.asarray(h_center, dtype=np.float32)
    h_neighbors = np.asarray(h_neighbors, dtype=np.float32)
    differences = np.asarray(differences, dtype=np.float32)

    wf1 = np.ascontiguousarray(np.asarray(W_f1, np.float16))    # [4, W]
    wf2 = np.ascontiguousarray(np.asarray(W_f2, np.float16)).reshape(2, 128, W)
    wnb = np.ascontiguousarray(np.asarray(W_nb, np.float16)).reshape(2, 128, W)
    wc = np.ascontiguousarray(np.asarray(W_c, np.float16)).reshape(2, 128, W)
    bf1 = np.asarray(b_f1, np.float32).reshape(2, 128, 1)
    bf2 = np.asarray(b_f2, np.float32).reshape(2, 128, 1)
    bnb = np.asarray(b_nb, np.float32).reshape(2, 128, 1)
    bc = np.asarray(b_c, np.float32).reshape(2, 128, 1)

    in_maps = []
    for c in range(NCORES):
        rs = slice(c * NC_ROWS, (c + 1) * NC_ROWS)
        xt = np.ascontiguousarray(
            h_neighbors[rs].reshape(E, W).T.astype(np.float16)).reshape(2, 128, E)
        dt_ = np.ascontiguousarray(differences[rs].reshape(E, F).T.astype(np.float16))
        ct = np.ascontiguousarray(h_center[rs].T.astype(np.float16)).reshape(2, 128, NC_ROWS)
        in_maps.append(dict(xt=xt, dt=dt_, ct=ct, wf1=wf1, wf2=wf2, wnb=wnb,
                            wc=wc, bf1=bf1, bf2=bf2, bnb=bnb, bc=bc))

    global _last_in_maps
    _last_in_maps = in_maps
    nc = _get_nc()
    res = run_bass_kernel_spmd(nc, in_maps, list(range(NCORES)))

    out = np.empty((N, W), np.float32)
    for c in range(NCORES):
        rs = slice(c * NC_ROWS, (c + 1) * NC_ROWS)
        out[rs] = res.results[c]["out_t"].reshape(W, NC_ROWS).T
    return out
